# revision 37
# baseline (speedup 1.0000x reference)
"""GatedDeltaNet on 8 trn2 NeuronCores (Bass/Tile).

Sharding: 2 full heads (DV=256) per core; 12 real heads on 8 cores means
cores 4-7 carry one real head plus a zero-output dummy slot (its Wo rows are
zero, so the final ReduceScatter ignores it). Per-core work:
  - projections hs @ [Wq|Wk|Wv|Wg|Wb|Wa] slices (feature-major via hsT)
  - causal 4-tap conv + SiLU, batched l2norm (q,k)
  - chunked gated delta rule, chunk C=128: per (chunk, head) precompute
    (I+A)^-1 via Neumann squaring (A nilpotent strict-lower), fused so each
    level is 2 matmuls; then 2 sequential state matmuls per chunk.
  - RMS-norm is fully local (whole DV on one core) - no norm collective.
  - per-chunk o_proj partial [128, HID] + per-chunk ReduceScatter(+).
Host only slices/casts weights and interleaves the 8 output row-blocks.
"""

import numpy as np
import ml_dtypes
from contextlib import ExitStack

import concourse.bass as bass
import concourse.bacc as bacc
import concourse.mybir as mybir
import concourse.tile as tile
from concourse.bass_utils import run_bass_kernel_spmd

F32 = mybir.dt.float32
BF16 = mybir.dt.bfloat16
AF = mybir.ActivationFunctionType
OP = mybir.AluOpType
BF = ml_dtypes.bfloat16

T, HID = 1024, 2048
H, DK, DV = 12, 128, 256
C, NCH = 128, 8          # chunk size, num chunks
NH = 2                   # head slots per core (full DV each)
NCORES = 8
WCOLS = 1540             # q 0:256 | k 256:512 | v 512:1024 | g 1024:1536 | b 2 | a 2
NEG = -1e30
EPS_L2 = 1e-6
NORM_EPS = 1e-5
RSROWS = T // NCORES // NCH   # 16 rows per core per chunk ReduceScatter


def build():
    nc = bacc.Bacc("TRN2", debug=False, num_devices=NCORES)
    d_hsT = nc.dram_tensor("hsT", [HID, T], BF16, kind="ExternalInput").ap()
    d_w = nc.dram_tensor("w", [HID, WCOLS], BF16, kind="ExternalInput").ap()
    d_wo = nc.dram_tensor("wo", [NH * DV, HID], BF16, kind="ExternalInput").ap()
    d_convw = nc.dram_tensor("convw", [1024, 4], F32, kind="ExternalInput").ap()
    d_masks = nc.dram_tensor("masks", [128, 256], F32, kind="ExternalInput").ap()
    d_identb = nc.dram_tensor("identb", [128, 128], BF16, kind="ExternalInput").ap()
    d_identf = nc.dram_tensor("identf", [128, 128], F32, kind="ExternalInput").ap()
    d_onesf = nc.dram_tensor("onesf", [1, 128], F32, kind="ExternalInput").ap()
    d_scal = nc.dram_tensor("scal", [NH, 2], F32, kind="ExternalInput").ap()
    d_out = nc.dram_tensor("out", [T // NCORES, HID], BF16, kind="ExternalOutput").ap()

    with tile.TileContext(nc) as tc, ExitStack() as ctx:
        def pool(name, bufs, space="SBUF"):
            return ctx.enter_context(tc.tile_pool(name=name, bufs=bufs, space=space))

        konst = pool("konst", 1)
        p_hsT = pool("p_hsT", 16)
        p_wk = pool("p_wk", 24)
        p_ps = pool("p_ps", 5, space="PSUM")    # unified [128,<=512] psum
        p_pf = pool("p_pf", 3, space="PSUM")    # [128,<=384] fused-rhs psum
        p_x = pool("p_x", 3)
        p_xs = pool("p_xs", 8)
        p_g = pool("p_g", 1)
        p_small = pool("p_small", 1)
        p_tm = pool("p_tm", 8)
        p_raw = pool("p_raw", 8)
        p_sc = pool("p_sc", 2)
        p_per = pool("p_per", 6)
        p_chain = pool("p_chain", 2)
        p_out = pool("p_out", 2)
        p_dram = pool("p_dram", 1, space="DRAM")

        # ---- constants ----
        masks = konst.tile([128, 256], F32, tag="masks")
        nc.sync.dma_start(masks, d_masks)
        mask_a, mask_m = masks[:, 0:128], masks[:, 128:256]
        identb = konst.tile([128, 128], BF16, tag="identb")
        nc.sync.dma_start(identb, d_identb)
        identf = konst.tile([128, 128], F32, tag="identf")
        nc.sync.dma_start(identf, d_identf)
        onesf = konst.tile([1, 128], F32, tag="onesf")
        nc.sync.dma_start(onesf, d_onesf)
        scal = konst.tile([NH, 2], F32, tag="scal")
        nc.sync.dma_start(scal, d_scal)
        convw = konst.tile([128, 8, 4], F32, tag="convw")
        nc.sync.dma_start(convw, d_convw.rearrange("(n p) k -> p n k", p=128))
        zeros2 = konst.tile([NH, 128], F32, tag="zeros2")
        nc.gpsimd.memset(zeros2, 0.0)
        epsl2 = konst.tile([128, 1], F32, tag="epsl2")
        nc.gpsimd.memset(epsl2, EPS_L2)
        epsn = konst.tile([128, 1], F32, tag="epsn")
        nc.gpsimd.memset(epsn, NORM_EPS)

        # Wo tiles (needed late; DMA overlaps)
        wo_sb = [konst.tile([128, HID], BF16, tag=f"wo{j}", name=f"wo{j}")
                 for j in range(4)]
        for j in range(4):
            nc.sync.dma_start(wo_sb[j], d_wo[j * 128:(j + 1) * 128, :])

        # ---- load hsT ----
        hsT = []
        for kt in range(16):
            t_ = p_hsT.tile([128, T], BF16, tag="hsT")
            nc.sync.dma_start(t_, d_hsT[kt * 128:(kt + 1) * 128, :])
            hsT.append(t_)

        # ---- projections (feature-major) + conv + silu ----
        # ct 0-1 q, 2-3 k, 4-7 v, 8-11 g, 12 b(2), 13 a(2)
        X = [None] * 8
        XS = [None] * 8
        G = [None] * 4
        zb = p_small.tile([NH, T], F32, tag="zb")
        za = p_small.tile([NH, T], F32, tag="za")
        for ct in [12, 13, 0, 1, 2, 3, 4, 5, 6, 7, 8, 9, 10, 11]:
            c0 = ct * 128 if ct < 13 else 1538
            cw = 128 if ct < 12 else 2
            wts = []
            for kt in range(16):
                wt = p_wk.tile([128, 128], BF16, tag="wk")
                nc.sync.dma_start(wt[:, 0:cw], d_w[kt * 128:(kt + 1) * 128, c0:c0 + cw])
                wts.append(wt)
            if ct < 8:
                X[ct] = p_x.tile([128, T + 4], BF16, tag="x", name=f"x{ct}")
                nc.gpsimd.memset(X[ct][:, 0:4], 0.0)
            elif ct < 12:
                G[ct - 8] = p_g.tile([128, T], BF16, tag=f"g{ct - 8}", name=f"g{ct - 8}")
            # kt-outer / th-inner: consecutive matmuls share lhsT -> 1 LDW / 2 MM
            ps0 = p_ps.tile([cw, 512], F32, tag="pp", name="ps0")
            ps1 = p_ps.tile([cw, 512], F32, tag="pp", name="ps1")
            for kt in range(16):
                nc.tensor.matmul(ps0, wts[kt][:, 0:cw], hsT[kt][:, 0:512],
                                 start=(kt == 0), stop=(kt == 15))
                nc.tensor.matmul(ps1, wts[kt][:, 0:cw], hsT[kt][:, 512:1024],
                                 start=(kt == 0), stop=(kt == 15))
            for th, ps in ((0, ps0), (1, ps1)):
                if ct < 8:
                    nc.scalar.copy(X[ct][:, 4 + th * 512: 4 + (th + 1) * 512], ps)
                elif ct < 12:
                    nc.scalar.activation(G[ct - 8][:, th * 512:(th + 1) * 512], ps, AF.Silu)
                elif ct == 12:
                    nc.scalar.copy(zb[:, th * 512:(th + 1) * 512], ps)
                else:
                    nc.scalar.copy(za[:, th * 512:(th + 1) * 512], ps)
            if ct < 8:
                # conv: keep a 1-shifted copy so all taps hit 4B-aligned offsets
                w4 = convw[:, ct]                      # [128, 4] f32
                xo = p_sc.tile([128, T + 2], BF16, tag="conv_o", name=f"xo{ct}")
                nc.vector.tensor_copy(xo, X[ct][:, 1:T + 3])
                ya = p_sc.tile([128, T], BF16, tag="conv_a", name=f"ya{ct}")
                yb = p_sc.tile([128, T], BF16, tag="conv_b", name=f"yb{ct}")
                nc.vector.tensor_scalar_mul(ya, xo[:, 0:T], w4[:, 0:1])
                nc.vector.scalar_tensor_tensor(yb, X[ct][:, 2:T + 2], w4[:, 1:2], ya,
                                               op0=OP.mult, op1=OP.add)
                nc.vector.scalar_tensor_tensor(ya, xo[:, 2:T + 2], w4[:, 2:3], yb,
                                               op0=OP.mult, op1=OP.add)
                nc.vector.scalar_tensor_tensor(yb, X[ct][:, 4:T + 4], w4[:, 3:4], ya,
                                               op0=OP.mult, op1=OP.add)
                XS[ct] = p_xs.tile([128, T], BF16, tag="xs", name=f"xs{ct}")
                nc.scalar.activation(XS[ct], yb, AF.Silu)

        # ---- beta / g / per-chunk cumsum (feature-major, base-0) ----
        beta_fm = p_small.tile([NH, T], F32, tag="beta_fm")
        nc.scalar.activation(beta_fm, zb, AF.Exp, scale=-1.0)
        nc.vector.tensor_scalar_add(beta_fm, beta_fm, 1.0)
        nc.vector.reciprocal(beta_fm, beta_fm)
        sp = p_small.tile([NH, T], F32, tag="sp")
        nc.scalar.activation(sp, za, AF.Exp, bias=scal[:, 1:2])
        nc.vector.tensor_scalar_add(sp, sp, 1.0)
        nc.scalar.activation(sp, sp, AF.Ln)
        nc.vector.tensor_scalar_mul(sp, sp, scal[:, 0:1])
        cum2 = p_small.tile([NH, T], F32, tag="cum2")
        for c in range(NCH):
            sl = slice(c * C, (c + 1) * C)
            nc.vector.tensor_tensor_scan(cum2[:, sl], sp[:, sl], zeros2,
                                         initial=0.0, op0=OP.add, op1=OP.add)
        # per-(chunk, head) cum rows at base 0, laid out (c, j)
        cumcat = p_small.tile([1, NCH * NH * 128], F32, tag="cumcat")
        for c in range(NCH):
            for j in range(NH):
                nc.sync.dma_start(cumcat[0:1, (c * NH + j) * 128:(c * NH + j + 1) * 128],
                                  cum2[j:j + 1, c * C:(c + 1) * C])

        # ---- transposes to token-major per chunk + l2 sumsq ----
        vtm, gtm, bctm, gam = [], [], [], []
        qraw, kraw = [], []
        ssq32 = p_small.tile([128, NCH * 4], F32, tag="ssq32")
        for c in range(NCH):
            sl = slice(c * C, (c + 1) * C)
            qt = p_raw.tile([128, NH * 128], BF16, tag="qtm")
            kt_ = p_raw.tile([128, NH * 128], BF16, tag="ktm")
            vt = p_tm.tile([128, NH * 256], BF16, tag="vtm")
            gt_ = p_tm.tile([128, NH * 256], BF16, tag="gtm")
            for jj in range(2):
                ps = p_ps.tile([128, 128], BF16, tag="pp", name="tps")
                nc.tensor.transpose(ps, XS[jj][:, sl], identb)
                nc.vector.tensor_copy(qt[:, jj * 128:(jj + 1) * 128], ps)
                ps = p_ps.tile([128, 128], BF16, tag="pp", name="tps2")
                nc.tensor.transpose(ps, XS[2 + jj][:, sl], identb)
                nc.vector.tensor_copy(kt_[:, jj * 128:(jj + 1) * 128], ps)
            for jj in range(4):
                ps = p_ps.tile([128, 128], BF16, tag="pp", name="tps3")
                nc.tensor.transpose(ps, XS[4 + jj][:, sl], identb)
                nc.vector.tensor_copy(vt[:, jj * 128:(jj + 1) * 128], ps)
                ps = p_ps.tile([128, 128], BF16, tag="pp", name="tps4")
                nc.tensor.transpose(ps, G[jj][:, sl], identb)
                nc.vector.tensor_copy(gt_[:, jj * 128:(jj + 1) * 128], ps)
            vtm.append(vt); gtm.append(gt_)
            qraw.append(qt); kraw.append(kt_)
            bt = p_tm.tile([128, 4], F32, tag="bctm")
            psb = p_ps.tile([128, 2], F32, tag="pp", name="psb")
            nc.tensor.transpose(psb, beta_fm[:, sl], identf[0:2, 0:2])
            nc.vector.tensor_copy(bt[:, 0:2], psb)
            psc = p_ps.tile([128, 2], F32, tag="pp", name="psc")
            nc.tensor.transpose(psc, cum2[:, sl], identf[0:2, 0:2])
            nc.vector.tensor_copy(bt[:, 2:4], psc)
            bctm.append(bt)
            ga = p_tm.tile([128, NH], F32, tag="gam")
            nc.scalar.activation(ga, bt[:, 2:4], AF.Exp)
            gam.append(ga)
            # sumsq cols: c*4 + (q0, q1, k0, k1)
            for i, (raw, jj) in enumerate([(qt, 0), (qt, 1), (kt_, 0), (kt_, 1)]):
                scr = p_sc.tile([128, 128], BF16, tag="sq_scr")
                nc.scalar.activation(scr, raw[:, jj * 128:(jj + 1) * 128], AF.Square,
                                     accum_out=ssq32[:, c * 4 + i:c * 4 + i + 1])

        # batched l2norm factors, then scale q/k in place
        lfac = p_small.tile([128, NCH * 4], F32, tag="lfac")
        nc.scalar.activation(lfac, ssq32, AF.Ln, bias=epsl2)
        rfac = p_small.tile([128, NCH * 4], F32, tag="rfac")
        nc.scalar.activation(rfac, lfac, AF.Exp, scale=-0.5)
        qn, kn = qraw, kraw
        for c in range(NCH):
            for j in range(NH):
                sl2 = slice(j * 128, (j + 1) * 128)
                nc.vector.tensor_scalar_mul(qraw[c][:, sl2], qraw[c][:, sl2],
                                            rfac[:, c * 4 + j:c * 4 + j + 1])
                nc.vector.tensor_scalar_mul(kraw[c][:, sl2], kraw[c][:, sl2],
                                            rfac[:, c * 4 + 2 + j:c * 4 + 3 + j])

        # ---- delta rule: per-(chunk, head) precompute + chains + norm + out ----
        S = []
        for j in range(NH):
            s_t = p_chain.tile([128, DV], BF16, tag=f"S{j}")
            nc.gpsimd.memset(s_t, 0.0)
            S.append(s_t)

        op_in = p_dram.tile([T, HID], BF16, tag="op_in")
        rs_out = p_dram.tile([T // NCORES, HID], BF16, tag="rs_out")

        for c in range(NCH):
            # E broadcast for both heads in one K=1 matmul
            e_ps = p_ps.tile([128, NH * 128], F32, tag="pp", name="e_ps")
            nc.tensor.matmul(e_ps, onesf,
                             cumcat[0:1, c * NH * 128:(c + 1) * NH * 128],
                             start=True, stop=True)
            per_j = []
            for j in range(NH):
                sl2 = slice(j * 128, (j + 1) * 128)
                slv = slice(j * 256, (j + 1) * 256)
                gamma_col = gam[c][:, j:j + 1]
                beta_col = bctm[c][:, j:j + 1]
                cum_col = bctm[c][:, 2 + j:3 + j]

                gq = p_sc.tile([128, 128], BF16, tag="gq")
                nc.vector.tensor_scalar_mul(gq, qn[c][:, sl2], gamma_col)
                # gkv = [gamma*K | V] for the fused TK|W1 matmul
                gkv = p_sc.tile([128, 384], BF16, tag="gkv")
                nc.vector.tensor_scalar_mul(gkv[:, 0:128], kn[c][:, sl2], gamma_col)
                nc.gpsimd.tensor_copy(gkv[:, 128:384], vtm[c][:, slv])
                kb = p_sc.tile([128, 128], BF16, tag="kb")
                nc.vector.tensor_scalar_mul(kb, kn[c][:, sl2], beta_col)

                qk_ps = p_ps.tile([128, 512], BF16, tag="pp", name="qkps")
                nc.tensor.transpose(qk_ps[:, 0:128], qn[c][:, sl2], identb)
                nc.tensor.transpose(qk_ps[:, 128:256], kn[c][:, sl2], identb)
                nc.tensor.transpose(qk_ps[:, 256:384], kb, identb)
                nc.tensor.transpose(qk_ps[:, 384:512], gq, identb)
                qkfm = p_sc.tile([128, 512], BF16, tag="qkfm")
                nc.vector.tensor_copy(qkfm, qk_ps)
                q_fm, k_fm = qkfm[:, 0:128], qkfm[:, 128:256]
                kb_fm, gq_fm = qkfm[:, 256:384], qkfm[:, 384:512]

                kkq_ps = p_ps.tile([128, 256], F32, tag="pp", name="kkq")
                nc.tensor.matmul(kkq_ps[:, 0:128], k_fm, kb_fm, start=True, stop=True)
                nc.tensor.matmul(kkq_ps[:, 128:256], kb_fm, q_fm, start=True, stop=True)
                kk_ps = kkq_ps[:, 0:128]
                kq_ps = kkq_ps[:, 128:256]

                # x_am = (cum_j - cum_i) - mask_a ; ea = exp(-x_am)
                x_am = p_sc.tile([128, 128], F32, tag="x_am")
                nc.vector.scalar_tensor_tensor(x_am, e_ps[:, sl2], cum_col, mask_a,
                                               op0=OP.subtract, op1=OP.subtract)
                ea = p_sc.tile([128, 128], F32, tag="ea")
                nc.scalar.activation(ea, x_am, AF.Exp, scale=-1.0)
                xm = p_sc.tile([128, 128], F32, tag="xm")
                nc.vector.scalar_tensor_tensor(xm, e_ps[:, sl2], cum_col, mask_m,
                                               op0=OP.subtract, op1=OP.add)
                em = p_sc.tile([128, 128], F32, tag="em")
                nc.scalar.activation(em, xm, AF.Exp)

                B_ = p_sc.tile([128, 128], BF16, tag="B")
                nc.vector.scalar_tensor_tensor(B_, ea, -1.0, kk_ps,
                                               op0=OP.mult, op1=OP.mult)
                # mtku = [Mt | Ku]
                mtku = p_per.tile([128, 256], BF16, tag="mtku")
                nc.vector.tensor_mul(mtku[:, 0:128], em, kq_ps)
                s_col = p_sc.tile([128, 1], F32, tag="s_col")
                nc.gpsimd.tensor_mul(s_col, beta_col, em[:, 127:128])
                nc.vector.tensor_scalar_mul(mtku[:, 128:256], kn[c][:, sl2], s_col)
                gc_col = p_sc.tile([128, 1], F32, tag="gc_col")
                nc.gpsimd.tensor_mul(gc_col, em[:, 127:128], gamma_col)

                # Neumann squaring, 7 factors (I+B^(2^L)); pr = [Bp^T | Rt]
                bt_ps = p_ps.tile([128, 128], BF16, tag="pp", name="btps")
                nc.tensor.transpose(bt_ps, B_, identb)
                pr = p_sc.tile([128, 256], BF16, tag="pr0")
                nc.vector.tensor_copy(pr[:, 0:128], bt_ps)
                nc.gpsimd.tensor_copy(pr[:, 128:256], identb)
                bp = B_
                for lvl in range(7):
                    bpt, rt = pr[:, 0:128], pr[:, 128:256]
                    if lvl < 6:
                        b2 = p_ps.tile([128, 128], F32, tag="pp")
                        nc.tensor.matmul(b2, bpt, bp, start=True, stop=True)
                        prp = p_pf.tile([128, 256], F32, tag="pf", name="prp")
                        nc.tensor.matmul(prp, bp, pr, start=True, stop=True)
                        pr2 = p_sc.tile([128, 256], BF16, tag=f"pr{lvl + 1}")
                        nc.scalar.copy(pr2[:, 0:128], prp[:, 0:128])
                        nc.vector.tensor_add(pr2[:, 128:256], rt, prp[:, 128:256])
                        bp2 = p_sc.tile([128, 128], BF16, tag=f"bp{lvl}")
                        nc.vector.tensor_copy(bp2, b2)
                        pr, bp = pr2, bp2
                    else:
                        pps = p_ps.tile([128, 128], F32, tag="pp")
                        nc.tensor.matmul(pps, bp, rt, start=True, stop=True)
                        rt2 = p_sc.tile([128, 128], BF16, tag="rt7")
                        nc.vector.tensor_add(rt2, rt, pps)
                tinvT = rt2

                # fused [TK | W1] = Tinv @ [gamma K | V]
                tkw_ps = p_pf.tile([128, 384], F32, tag="pf", name="tkwp")
                nc.tensor.matmul(tkw_ps, tinvT, gkv, start=True, stop=True)
                tkw = p_per.tile([128, 384], BF16, tag="tkw")
                nc.vector.tensor_copy(tkw, tkw_ps)
                tk, w1 = tkw[:, 0:128], tkw[:, 128:384]
                # fused TK^T @ [Mt | Ku]
                mg_ps = p_pf.tile([128, 256], F32, tag="pf", name="mgp")
                nc.tensor.matmul(mg_ps, tk, mtku, start=True, stop=True)
                qeffT = p_per.tile([128, 128], BF16, tag="qeffT")
                nc.vector.tensor_sub(qeffT, gq_fm, mg_ps[:, 0:128])
                gt2 = p_per.tile([128, 128], BF16, tag="gt2")
                nc.vector.scalar_tensor_tensor(gt2, identb, gc_col, mg_ps[:, 128:256],
                                               op0=OP.mult, op1=OP.subtract)
                per_j.append((qeffT, gt2, mtku, w1))

            # chains + local rms-norm + gate for chunk c
            ssq_c = p_sc.tile([128, NH], F32, tag="ssq_c")
            o_pss = []
            for j in range(NH):
                qeffT, gt2, mtku, w1 = per_j[j]
                os_ps = p_ps.tile([128, 2 * DV], F32, tag="pp", name="osps")
                o_ps, s_ps = os_ps[:, 0:DV], os_ps[:, DV:2 * DV]
                nc.tensor.matmul(o_ps, qeffT, S[j], start=True, stop=False)
                nc.tensor.matmul(o_ps, mtku[:, 0:128], w1, start=False, stop=True)
                nc.tensor.matmul(s_ps, gt2, S[j], start=True, stop=False)
                nc.tensor.matmul(s_ps, mtku[:, 128:256], w1, start=False, stop=True)
                s_new = p_chain.tile([128, DV], BF16, tag=f"S{j}")
                nc.vector.tensor_copy(s_new, s_ps)
                S[j] = s_new
                scr = p_sc.tile([128, DV], BF16, tag="sq_scr2")
                nc.scalar.activation(scr, o_ps, AF.Square,
                                     accum_out=ssq_c[:, j:j + 1])
                o_pss.append(o_ps)
            nln = p_sc.tile([128, NH], F32, tag="nln")
            nc.scalar.activation(nln, ssq_c, AF.Ln, scale=1.0 / DV, bias=epsn)
            nrf = p_sc.tile([128, NH], F32, tag="nrf")
            nc.scalar.activation(nrf, nln, AF.Exp, scale=-0.5)
            ogT = []
            for j in range(NH):
                o_n = p_sc.tile([128, DV], BF16, tag="o_n")
                nc.vector.tensor_scalar_mul(o_n, o_pss[j], nrf[:, j:j + 1])
                o_g = p_sc.tile([128, DV], BF16, tag="o_g")
                nc.vector.tensor_mul(o_g, o_n, gtm[c][:, j * 256:(j + 1) * 256])
                for hh in range(2):
                    ps = p_ps.tile([128, 128], BF16, tag="pp", name="ogps")
                    nc.tensor.transpose(ps, o_g[:, hh * 128:(hh + 1) * 128], identb)
                    og = p_sc.tile([128, 128], BF16, tag=f"ogT{j}{hh}")
                    nc.vector.tensor_copy(og, ps)
                    ogT.append(og)
            outp = p_out.tile([128, HID], BF16, tag="outp")
            for nt in range(4):
                ps = p_ps.tile([128, 512], F32, tag="pp")
                for kk_ in range(4):
                    nc.tensor.matmul(ps, ogT[kk_], wo_sb[kk_][:, nt * 512:(nt + 1) * 512],
                                     start=(kk_ == 0), stop=(kk_ == 3))
                nc.scalar.copy(outp[:, nt * 512:(nt + 1) * 512], ps)
            nc.sync.dma_start(op_in[c * 128:(c + 1) * 128, :], outp)
            # staged ReduceScatter: chunks 0-5 overlap the remaining compute,
            # chunks 6-7 form a short tail
            if c == 5:
                nc.gpsimd.collective_compute(
                    "ReduceScatter", OP.add,
                    replica_groups=[list(range(NCORES))],
                    ins=[op_in[0:768, :].opt()],
                    outs=[rs_out[0:96, :].opt()],
                )
            elif c == 7:
                nc.gpsimd.collective_compute(
                    "ReduceScatter", OP.add,
                    replica_groups=[list(range(NCORES))],
                    ins=[op_in[768:1024, :].opt()],
                    outs=[rs_out[96:128, :].opt()],
                )
        nc.sync.dma_start(d_out, rs_out[:])

    nc.compile()
    return nc


_BUILT = None


def _get_built():
    global _BUILT
    if _BUILT is None:
        _BUILT = build()
    return _BUILT


def _prep_in_maps(hidden_states, Wq, Wk, Wv, Wb, Wa, Wg, Wo,
                  conv_wq, conv_wk, conv_wv, A_log, dt_bias, norm_w):
    hsT = np.ascontiguousarray(hidden_states[0].astype(np.float32).T).astype(BF)
    Wq_b, Wk_b = Wq.astype(BF), Wk.astype(BF)
    Wv_b, Wg_b = Wv.astype(BF), Wg.astype(BF)
    Wb_b, Wa_b = Wb.astype(BF), Wa.astype(BF)
    Wo_eff = (np.tile(norm_w.astype(np.float32), H)[:, None] * Wo).astype(BF)
    cq = conv_wq.astype(np.float32)
    ck = conv_wk.astype(np.float32)
    cv = conv_wv.astype(np.float32)

    ii, jj = np.indices((128, 128))
    mask_a = np.where(jj < ii, 0.0, NEG).astype(np.float32)   # strict lower keep
    mask_m = np.where(jj >= ii, 0.0, NEG).astype(np.float32)  # upper incl diag keep
    masks = np.ascontiguousarray(np.concatenate([mask_a, mask_m], axis=1))
    identb = np.eye(128, dtype=np.float32).astype(BF)
    identf = np.eye(128, dtype=np.float32)
    onesf = np.ones((1, 128), np.float32)

    in_maps = []
    for core in range(NCORES):
        if core < 4:
            heads = [2 * core, 2 * core + 1]
            real = [True, True]
        else:
            heads = [4 + core, 4 + core]      # second slot is a dummy copy
            real = [True, False]
        qk_idx = np.concatenate([np.arange(h * DK, (h + 1) * DK) for h in heads])
        v_idx = np.concatenate([np.arange(h * DV, (h + 1) * DV) for h in heads])
        w = np.concatenate([Wq_b[:, qk_idx], Wk_b[:, qk_idx],
                            Wv_b[:, v_idx], Wg_b[:, v_idx],
                            Wb_b[:, heads], Wa_b[:, heads]], axis=1)
        convw = np.ascontiguousarray(
            np.concatenate([cq[qk_idx], ck[qk_idx], cv[v_idx]], axis=0))
        scal = np.stack([-np.exp(A_log[heads].astype(np.float32)),
                         dt_bias[heads].astype(np.float32)], axis=1)
        wo = Wo_eff[v_idx].copy()
        for j in range(NH):
            if not real[j]:
                wo[j * DV:(j + 1) * DV] = 0
        in_maps.append({
            "hsT": hsT, "w": np.ascontiguousarray(w),
            "wo": np.ascontiguousarray(wo),
            "convw": convw, "masks": masks, "identb": identb,
            "identf": identf, "onesf": onesf,
            "scal": np.ascontiguousarray(scal),
        })
    return in_maps


def _run(in_maps, trace=False):
    nc = _get_built()
    return run_bass_kernel_spmd(nc, in_maps, list(range(NCORES)), trace=trace)


def _assemble(res):
    # two ReduceScatter regions: rows 0:768 (96 rows/core) and 768:1024
    # (32 rows/core)
    out = np.zeros((T, HID), np.float32)
    for r in range(NCORES):
        o = np.asarray(res.results[r]["out"]).astype(np.float32)
        out[r * 96:(r + 1) * 96] = o[0:96]
        out[768 + r * 32:768 + (r + 1) * 32] = o[96:128]
    return out.reshape(1, T, HID)


def kernel(**inputs):
    in_maps = _prep_in_maps(**inputs)
    res = _run(in_maps, trace=False)
    return _assemble(res)


def _ensure_ntff_hook():
    import sys as _sys
    import types as _types
    try:
        from antenv.axon_hooks import get_axon_ntff_profile_hook  # noqa: F401
        return
    except ImportError:
        pass
    from trn_agent_boot.trn_boot import _ntff_profile_via_ctypes
    hook = _ntff_profile_via_ctypes('/opt/axon/libaxon_pjrt.so')
    mod = _types.ModuleType("antenv.axon_hooks")
    mod.get_axon_ntff_profile_hook = lambda: hook
    mod.set_axon_ntff_profile_hook = lambda h: None
    _sys.modules["antenv.axon_hooks"] = mod
    import antenv
    antenv.axon_hooks = mod
    import concourse.bass_utils as _bu
    _bu.upload_artifacts = lambda d: ""


def kernel_traced(**inputs):
    _ensure_ntff_hook()
    in_maps = _prep_in_maps(**inputs)
    res = _run(in_maps, trace=True)
    return _assemble(res), res


# revision 38
# speedup vs baseline: 1.0459x; 1.0459x over previous
"""GatedDeltaNet on 8 trn2 NeuronCores (Bass/Tile).

Sharding: 2 full heads (DV=256) per core; 12 real heads on 8 cores means
cores 4-7 carry one real head plus a zero-output dummy slot (its Wo rows are
zero, so the final ReduceScatter ignores it). Per-core work:
  - projections hs @ [Wq|Wk|Wv|Wg|Wb|Wa] slices (feature-major via hsT)
  - causal 4-tap conv + SiLU, batched l2norm (q,k)
  - chunked gated delta rule, chunk C=128: per (chunk, head) precompute
    (I+A)^-1 via Neumann squaring (A nilpotent strict-lower), fused so each
    level is 2 matmuls; then 2 sequential state matmuls per chunk.
  - RMS-norm is fully local (whole DV on one core) - no norm collective.
  - per-chunk o_proj partial [128, HID] + per-chunk ReduceScatter(+).
Host only slices/casts weights and interleaves the 8 output row-blocks.
"""

import numpy as np
import ml_dtypes
from contextlib import ExitStack

import concourse.bass as bass
import concourse.bacc as bacc
import concourse.mybir as mybir
import concourse.tile as tile
from concourse.bass_utils import run_bass_kernel_spmd

F32 = mybir.dt.float32
BF16 = mybir.dt.bfloat16
AF = mybir.ActivationFunctionType
OP = mybir.AluOpType
BF = ml_dtypes.bfloat16

T, HID = 1024, 2048
H, DK, DV = 12, 128, 256
C, NCH = 128, 8          # chunk size, num chunks
NH = 2                   # head slots per core (full DV each)
NCORES = 8
WCOLS = 1540             # q 0:256 | k 256:512 | v 512:1024 | g 1024:1536 | b 2 | a 2
NEG = -1e30
EPS_L2 = 1e-6
NORM_EPS = 1e-5
RSROWS = T // NCORES // NCH   # 16 rows per core per chunk ReduceScatter


def build():
    nc = bacc.Bacc("TRN2", debug=False, num_devices=NCORES)
    d_hsT = nc.dram_tensor("hsT", [HID, T], BF16, kind="ExternalInput").ap()
    d_w = nc.dram_tensor("w", [HID, WCOLS], BF16, kind="ExternalInput").ap()
    d_wo = nc.dram_tensor("wo", [NH * DV, HID], BF16, kind="ExternalInput").ap()
    d_convw = nc.dram_tensor("convw", [1024, 4], F32, kind="ExternalInput").ap()
    d_masks = nc.dram_tensor("masks", [128, 256], F32, kind="ExternalInput").ap()
    d_identb = nc.dram_tensor("identb", [128, 128], BF16, kind="ExternalInput").ap()
    d_identf = nc.dram_tensor("identf", [128, 128], F32, kind="ExternalInput").ap()
    d_onesf = nc.dram_tensor("onesf", [1, 128], F32, kind="ExternalInput").ap()
    d_scal = nc.dram_tensor("scal", [NH, 2], F32, kind="ExternalInput").ap()
    d_out = nc.dram_tensor("out", [T // NCORES, HID], BF16, kind="ExternalOutput").ap()

    with tile.TileContext(nc) as tc, ExitStack() as ctx:
        def pool(name, bufs, space="SBUF"):
            return ctx.enter_context(tc.tile_pool(name=name, bufs=bufs, space=space))

        konst = pool("konst", 1)
        p_hsT = pool("p_hsT", 16)
        p_wk = pool("p_wk", 24)
        p_ps = pool("p_ps", 5, space="PSUM")    # unified [128,<=512] psum
        p_pf = pool("p_pf", 3, space="PSUM")    # [128,<=384] fused-rhs psum
        p_x = pool("p_x", 3)
        p_xs = pool("p_xs", 8)
        p_g = pool("p_g", 1)
        p_small = pool("p_small", 1)
        p_tm = pool("p_tm", 8)
        p_raw = pool("p_raw", 8)
        p_sc = pool("p_sc", 2)
        p_per = pool("p_per", 6)
        p_chain = pool("p_chain", 2)
        p_out = pool("p_out", 2)
        p_dram = pool("p_dram", 1, space="DRAM")

        # ---- constants ----
        masks = konst.tile([128, 256], F32, tag="masks")
        nc.sync.dma_start(masks, d_masks)
        mask_a, mask_m = masks[:, 0:128], masks[:, 128:256]
        identb = konst.tile([128, 128], BF16, tag="identb")
        nc.sync.dma_start(identb, d_identb)
        identf = konst.tile([128, 128], F32, tag="identf")
        nc.sync.dma_start(identf, d_identf)
        onesf = konst.tile([1, 128], F32, tag="onesf")
        nc.sync.dma_start(onesf, d_onesf)
        scal = konst.tile([NH, 2], F32, tag="scal")
        nc.sync.dma_start(scal, d_scal)
        convw = konst.tile([128, 8, 4], F32, tag="convw")
        nc.sync.dma_start(convw, d_convw.rearrange("(n p) k -> p n k", p=128))
        zeros2 = konst.tile([NH, 128], F32, tag="zeros2")
        nc.gpsimd.memset(zeros2, 0.0)
        epsl2 = konst.tile([128, 1], F32, tag="epsl2")
        nc.gpsimd.memset(epsl2, EPS_L2)
        epsn = konst.tile([128, 1], F32, tag="epsn")
        nc.gpsimd.memset(epsn, NORM_EPS)

        # Wo tiles (needed late; DMA overlaps)
        wo_sb = [konst.tile([128, HID], BF16, tag=f"wo{j}", name=f"wo{j}")
                 for j in range(4)]
        for j in range(4):
            nc.sync.dma_start(wo_sb[j], d_wo[j * 128:(j + 1) * 128, :])

        # ---- load hsT ----
        hsT = []
        for kt in range(16):
            t_ = p_hsT.tile([128, T], BF16, tag="hsT")
            nc.sync.dma_start(t_, d_hsT[kt * 128:(kt + 1) * 128, :])
            hsT.append(t_)

        # ---- projections (feature-major) + conv + silu ----
        # ct 0-1 q, 2-3 k, 4-7 v, 8-11 g, 12 b(2), 13 a(2)
        X = [None] * 8
        XS = [None] * 8
        G = [None] * 4
        zb = p_small.tile([NH, T], F32, tag="zb")
        za = p_small.tile([NH, T], F32, tag="za")
        for ct in [12, 13, 0, 1, 2, 3, 4, 5, 6, 7, 8, 9, 10, 11]:
            c0 = ct * 128 if ct < 13 else 1538
            cw = 128 if ct < 12 else 2
            wts = []
            for kt in range(16):
                wt = p_wk.tile([128, 128], BF16, tag="wk")
                nc.sync.dma_start(wt[:, 0:cw], d_w[kt * 128:(kt + 1) * 128, c0:c0 + cw])
                wts.append(wt)
            if ct < 8:
                X[ct] = p_x.tile([128, T + 4], BF16, tag="x", name=f"x{ct}")
                nc.gpsimd.memset(X[ct][:, 0:4], 0.0)
            elif ct < 12:
                G[ct - 8] = p_g.tile([128, T], BF16, tag=f"g{ct - 8}", name=f"g{ct - 8}")
            # kt-outer / th-inner: consecutive matmuls share lhsT -> 1 LDW / 2 MM
            ps0 = p_ps.tile([cw, 512], F32, tag="pp", name="ps0")
            ps1 = p_ps.tile([cw, 512], F32, tag="pp", name="ps1")
            for kt in range(16):
                nc.tensor.matmul(ps0, wts[kt][:, 0:cw], hsT[kt][:, 0:512],
                                 start=(kt == 0), stop=(kt == 15))
                nc.tensor.matmul(ps1, wts[kt][:, 0:cw], hsT[kt][:, 512:1024],
                                 start=(kt == 0), stop=(kt == 15))
            for th, ps in ((0, ps0), (1, ps1)):
                if ct < 8:
                    nc.scalar.copy(X[ct][:, 4 + th * 512: 4 + (th + 1) * 512], ps)
                elif ct < 12:
                    nc.scalar.activation(G[ct - 8][:, th * 512:(th + 1) * 512], ps, AF.Silu)
                elif ct == 12:
                    nc.scalar.copy(zb[:, th * 512:(th + 1) * 512], ps)
                else:
                    nc.scalar.copy(za[:, th * 512:(th + 1) * 512], ps)
            if ct < 8:
                # conv: keep a 1-shifted copy so all taps hit 4B-aligned offsets
                w4 = convw[:, ct]                      # [128, 4] f32
                xo = p_sc.tile([128, T + 2], BF16, tag="conv_o", name=f"xo{ct}")
                nc.vector.tensor_copy(xo, X[ct][:, 1:T + 3])
                ya = p_sc.tile([128, T], BF16, tag="conv_a", name=f"ya{ct}")
                yb = p_sc.tile([128, T], BF16, tag="conv_b", name=f"yb{ct}")
                nc.vector.tensor_scalar_mul(ya, xo[:, 0:T], w4[:, 0:1])
                nc.vector.scalar_tensor_tensor(yb, X[ct][:, 2:T + 2], w4[:, 1:2], ya,
                                               op0=OP.mult, op1=OP.add)
                nc.vector.scalar_tensor_tensor(ya, xo[:, 2:T + 2], w4[:, 2:3], yb,
                                               op0=OP.mult, op1=OP.add)
                nc.vector.scalar_tensor_tensor(yb, X[ct][:, 4:T + 4], w4[:, 3:4], ya,
                                               op0=OP.mult, op1=OP.add)
                XS[ct] = p_xs.tile([128, T], BF16, tag="xs", name=f"xs{ct}")
                nc.scalar.activation(XS[ct], yb, AF.Silu)

        # ---- beta / g / per-chunk cumsum (feature-major, base-0) ----
        beta_fm = p_small.tile([NH, T], F32, tag="beta_fm")
        nc.scalar.activation(beta_fm, zb, AF.Exp, scale=-1.0)
        nc.vector.tensor_scalar_add(beta_fm, beta_fm, 1.0)
        nc.vector.reciprocal(beta_fm, beta_fm)
        sp = p_small.tile([NH, T], F32, tag="sp")
        nc.scalar.activation(sp, za, AF.Exp, bias=scal[:, 1:2])
        nc.vector.tensor_scalar_add(sp, sp, 1.0)
        nc.scalar.activation(sp, sp, AF.Ln)
        nc.vector.tensor_scalar_mul(sp, sp, scal[:, 0:1])
        cum2 = p_small.tile([NH, T], F32, tag="cum2")
        for c in range(NCH):
            sl = slice(c * C, (c + 1) * C)
            nc.vector.tensor_tensor_scan(cum2[:, sl], sp[:, sl], zeros2,
                                         initial=0.0, op0=OP.add, op1=OP.add)
        # per-(chunk, head) cum rows at base 0, laid out (c, j)
        cumcat = p_small.tile([1, NCH * NH * 128], F32, tag="cumcat")
        for c in range(NCH):
            for j in range(NH):
                nc.sync.dma_start(cumcat[0:1, (c * NH + j) * 128:(c * NH + j + 1) * 128],
                                  cum2[j:j + 1, c * C:(c + 1) * C])

        # ---- transposes to token-major per chunk + l2 sumsq ----
        vtm, gtm, bctm, gam = [], [], [], []
        qraw, kraw = [], []
        ssq32 = p_small.tile([128, NCH * 4], F32, tag="ssq32")
        for c in range(NCH):
            sl = slice(c * C, (c + 1) * C)
            qt = p_raw.tile([128, NH * 128], BF16, tag="qtm")
            kt_ = p_raw.tile([128, NH * 128], BF16, tag="ktm")
            vt = p_tm.tile([128, NH * 256], BF16, tag="vtm")
            gt_ = p_tm.tile([128, NH * 256], BF16, tag="gtm")
            for jj in range(2):
                ps = p_ps.tile([128, 128], BF16, tag="pp", name="tps")
                nc.tensor.transpose(ps, XS[jj][:, sl], identb)
                nc.vector.tensor_copy(qt[:, jj * 128:(jj + 1) * 128], ps)
                ps = p_ps.tile([128, 128], BF16, tag="pp", name="tps2")
                nc.tensor.transpose(ps, XS[2 + jj][:, sl], identb)
                nc.vector.tensor_copy(kt_[:, jj * 128:(jj + 1) * 128], ps)
            for jj in range(4):
                ps = p_ps.tile([128, 128], BF16, tag="pp", name="tps3")
                nc.tensor.transpose(ps, XS[4 + jj][:, sl], identb)
                nc.vector.tensor_copy(vt[:, jj * 128:(jj + 1) * 128], ps)
                ps = p_ps.tile([128, 128], BF16, tag="pp", name="tps4")
                nc.tensor.transpose(ps, G[jj][:, sl], identb)
                nc.vector.tensor_copy(gt_[:, jj * 128:(jj + 1) * 128], ps)
            vtm.append(vt); gtm.append(gt_)
            qraw.append(qt); kraw.append(kt_)
            bt = p_tm.tile([128, 4], F32, tag="bctm")
            psb = p_ps.tile([128, 2], F32, tag="pp", name="psb")
            nc.tensor.transpose(psb, beta_fm[:, sl], identf[0:2, 0:2])
            nc.vector.tensor_copy(bt[:, 0:2], psb)
            psc = p_ps.tile([128, 2], F32, tag="pp", name="psc")
            nc.tensor.transpose(psc, cum2[:, sl], identf[0:2, 0:2])
            nc.vector.tensor_copy(bt[:, 2:4], psc)
            bctm.append(bt)
            ga = p_tm.tile([128, NH], F32, tag="gam")
            nc.scalar.activation(ga, bt[:, 2:4], AF.Exp)
            gam.append(ga)
            # sumsq cols: c*4 + (q0, q1, k0, k1)
            for i, (raw, jj) in enumerate([(qt, 0), (qt, 1), (kt_, 0), (kt_, 1)]):
                scr = p_sc.tile([128, 128], BF16, tag="sq_scr")
                nc.scalar.activation(scr, raw[:, jj * 128:(jj + 1) * 128], AF.Square,
                                     accum_out=ssq32[:, c * 4 + i:c * 4 + i + 1])

        # batched l2norm factors, then scale q/k in place
        lfac = p_small.tile([128, NCH * 4], F32, tag="lfac")
        nc.scalar.activation(lfac, ssq32, AF.Ln, bias=epsl2)
        rfac = p_small.tile([128, NCH * 4], F32, tag="rfac")
        nc.scalar.activation(rfac, lfac, AF.Exp, scale=-0.5)
        qn, kn = qraw, kraw
        for c in range(NCH):
            for j in range(NH):
                sl2 = slice(j * 128, (j + 1) * 128)
                nc.vector.tensor_scalar_mul(qraw[c][:, sl2], qraw[c][:, sl2],
                                            rfac[:, c * 4 + j:c * 4 + j + 1])
                nc.vector.tensor_scalar_mul(kraw[c][:, sl2], kraw[c][:, sl2],
                                            rfac[:, c * 4 + 2 + j:c * 4 + 3 + j])

        # ---- delta rule: per-(chunk, head) precompute + chains + norm + out ----
        S = []
        for j in range(NH):
            s_t = p_chain.tile([128, DV], BF16, tag=f"S{j}")
            nc.gpsimd.memset(s_t, 0.0)
            S.append(s_t)

        op_in = p_dram.tile([T, HID], BF16, tag="op_in")
        rs_out = p_dram.tile([T // NCORES, HID], BF16, tag="rs_out")

        for c in range(NCH):
            # E broadcast for both heads in one K=1 matmul
            e_ps = p_ps.tile([128, NH * 128], F32, tag="pp", name="e_ps")
            nc.tensor.matmul(e_ps, onesf,
                             cumcat[0:1, c * NH * 128:(c + 1) * NH * 128],
                             start=True, stop=True)
            per_j = []
            for j in range(NH):
                sl2 = slice(j * 128, (j + 1) * 128)
                slv = slice(j * 256, (j + 1) * 256)
                gamma_col = gam[c][:, j:j + 1]
                beta_col = bctm[c][:, j:j + 1]
                cum_col = bctm[c][:, 2 + j:3 + j]

                gq = p_sc.tile([128, 128], BF16, tag="gq")
                nc.vector.tensor_scalar_mul(gq, qn[c][:, sl2], gamma_col)
                # gkv = [gamma*K | V] for the fused TK|W1 matmul
                gkv = p_sc.tile([128, 384], BF16, tag="gkv")
                nc.vector.tensor_scalar_mul(gkv[:, 0:128], kn[c][:, sl2], gamma_col)
                nc.gpsimd.tensor_copy(gkv[:, 128:384], vtm[c][:, slv])
                kb = p_sc.tile([128, 128], BF16, tag="kb")
                nc.vector.tensor_scalar_mul(kb, kn[c][:, sl2], beta_col)

                def tr(src, tag, eng=None, ident=identb):
                    ps = p_ps.tile([128, 128], BF16, tag="pp", name="trps")
                    nc.tensor.transpose(ps, src, ident)
                    out = p_sc.tile([128, 128], BF16, tag=tag)
                    if eng == "act":
                        nc.scalar.copy(out, ps)
                    else:
                        nc.vector.tensor_copy(out, ps)
                    return out

                q_fm = tr(qn[c][:, sl2], "q_fm", eng="act")
                k_fm = tr(kn[c][:, sl2], "k_fm")
                kb_fm = tr(kb, "kb_fm")
                gq_fm = tr(gq, "gq_fm", eng="act")

                kk_ps = p_ps.tile([128, 128], F32, tag="pp")
                nc.tensor.matmul(kk_ps, k_fm, kb_fm, start=True, stop=True)
                kq_ps = p_ps.tile([128, 128], F32, tag="pp")
                nc.tensor.matmul(kq_ps, kb_fm, q_fm, start=True, stop=True)

                # x_am = (cum_j - cum_i) - mask_a ; ea = exp(-x_am)
                x_am = p_sc.tile([128, 128], F32, tag="x_am")
                nc.vector.scalar_tensor_tensor(x_am, e_ps[:, sl2], cum_col, mask_a,
                                               op0=OP.subtract, op1=OP.subtract)
                ea = p_sc.tile([128, 128], F32, tag="ea")
                nc.scalar.activation(ea, x_am, AF.Exp, scale=-1.0)
                xm = p_sc.tile([128, 128], F32, tag="xm")
                nc.vector.scalar_tensor_tensor(xm, e_ps[:, sl2], cum_col, mask_m,
                                               op0=OP.subtract, op1=OP.add)
                em = p_sc.tile([128, 128], F32, tag="em")
                nc.scalar.activation(em, xm, AF.Exp)

                B_ = p_sc.tile([128, 128], BF16, tag="B")
                nc.vector.scalar_tensor_tensor(B_, ea, -1.0, kk_ps,
                                               op0=OP.mult, op1=OP.mult)
                # mtku = [Mt | Ku]
                mtku = p_per.tile([128, 256], BF16, tag="mtku")
                nc.vector.tensor_mul(mtku[:, 0:128], em, kq_ps)
                s_col = p_sc.tile([128, 1], F32, tag="s_col")
                nc.gpsimd.tensor_mul(s_col, beta_col, em[:, 127:128])
                nc.vector.tensor_scalar_mul(mtku[:, 128:256], kn[c][:, sl2], s_col)
                gc_col = p_sc.tile([128, 1], F32, tag="gc_col")
                nc.gpsimd.tensor_mul(gc_col, em[:, 127:128], gamma_col)

                # Neumann squaring, 7 factors (I+B^(2^L)); pr = [Bp^T | Rt]
                bt_ps = p_ps.tile([128, 128], BF16, tag="pp", name="btps")
                nc.tensor.transpose(bt_ps, B_, identb)
                pr = p_sc.tile([128, 256], BF16, tag="pr0")
                nc.vector.tensor_copy(pr[:, 0:128], bt_ps)
                nc.gpsimd.tensor_copy(pr[:, 128:256], identb)
                bp = B_
                for lvl in range(7):
                    bpt, rt = pr[:, 0:128], pr[:, 128:256]
                    if lvl < 6:
                        b2 = p_ps.tile([128, 128], F32, tag="pp")
                        nc.tensor.matmul(b2, bpt, bp, start=True, stop=True)
                        prp = p_pf.tile([128, 256], F32, tag="pf", name="prp")
                        nc.tensor.matmul(prp, bp, pr, start=True, stop=True)
                        pr2 = p_sc.tile([128, 256], BF16, tag=f"pr{lvl + 1}")
                        nc.scalar.copy(pr2[:, 0:128], prp[:, 0:128])
                        nc.vector.tensor_add(pr2[:, 128:256], rt, prp[:, 128:256])
                        bp2 = p_sc.tile([128, 128], BF16, tag=f"bp{lvl}")
                        nc.vector.tensor_copy(bp2, b2)
                        pr, bp = pr2, bp2
                    else:
                        pps = p_ps.tile([128, 128], F32, tag="pp")
                        nc.tensor.matmul(pps, bp, rt, start=True, stop=True)
                        rt2 = p_sc.tile([128, 128], BF16, tag="rt7")
                        nc.vector.tensor_add(rt2, rt, pps)
                tinvT = rt2

                # fused [TK | W1] = Tinv @ [gamma K | V]
                tkw_ps = p_pf.tile([128, 384], F32, tag="pf", name="tkwp")
                nc.tensor.matmul(tkw_ps, tinvT, gkv, start=True, stop=True)
                tkw = p_per.tile([128, 384], BF16, tag="tkw")
                nc.vector.tensor_copy(tkw, tkw_ps)
                tk, w1 = tkw[:, 0:128], tkw[:, 128:384]
                # fused TK^T @ [Mt | Ku]
                mg_ps = p_pf.tile([128, 256], F32, tag="pf", name="mgp")
                nc.tensor.matmul(mg_ps, tk, mtku, start=True, stop=True)
                qeffT = p_per.tile([128, 128], BF16, tag="qeffT")
                nc.vector.tensor_sub(qeffT, gq_fm, mg_ps[:, 0:128])
                gt2 = p_per.tile([128, 128], BF16, tag="gt2")
                nc.vector.scalar_tensor_tensor(gt2, identb, gc_col, mg_ps[:, 128:256],
                                               op0=OP.mult, op1=OP.subtract)
                per_j.append((qeffT, gt2, mtku, w1))

            # chains + local rms-norm + gate for chunk c
            ssq_c = p_sc.tile([128, NH], F32, tag="ssq_c")
            o_pss = []
            for j in range(NH):
                qeffT, gt2, mtku, w1 = per_j[j]
                o_ps = p_ps.tile([128, DV], F32, tag="pp")
                nc.tensor.matmul(o_ps, qeffT, S[j], start=True, stop=False)
                nc.tensor.matmul(o_ps, mtku[:, 0:128], w1, start=False, stop=True)
                s_ps = p_ps.tile([128, DV], F32, tag="pp")
                nc.tensor.matmul(s_ps, gt2, S[j], start=True, stop=False)
                nc.tensor.matmul(s_ps, mtku[:, 128:256], w1, start=False, stop=True)
                s_new = p_chain.tile([128, DV], BF16, tag=f"S{j}")
                nc.vector.tensor_copy(s_new, s_ps)
                S[j] = s_new
                scr = p_sc.tile([128, DV], BF16, tag="sq_scr2")
                nc.scalar.activation(scr, o_ps, AF.Square,
                                     accum_out=ssq_c[:, j:j + 1])
                o_pss.append(o_ps)
            nln = p_sc.tile([128, NH], F32, tag="nln")
            nc.scalar.activation(nln, ssq_c, AF.Ln, scale=1.0 / DV, bias=epsn)
            nrf = p_sc.tile([128, NH], F32, tag="nrf")
            nc.scalar.activation(nrf, nln, AF.Exp, scale=-0.5)
            ogT = []
            for j in range(NH):
                o_n = p_sc.tile([128, DV], BF16, tag="o_n")
                nc.vector.tensor_scalar_mul(o_n, o_pss[j], nrf[:, j:j + 1])
                o_g = p_sc.tile([128, DV], BF16, tag="o_g")
                nc.vector.tensor_mul(o_g, o_n, gtm[c][:, j * 256:(j + 1) * 256])
                for hh in range(2):
                    ps = p_ps.tile([128, 128], BF16, tag="pp", name="ogps")
                    nc.tensor.transpose(ps, o_g[:, hh * 128:(hh + 1) * 128], identb)
                    og = p_sc.tile([128, 128], BF16, tag=f"ogT{j}{hh}")
                    nc.vector.tensor_copy(og, ps)
                    ogT.append(og)
            outp = p_out.tile([128, HID], BF16, tag="outp")
            for nt in range(4):
                ps = p_ps.tile([128, 512], F32, tag="pp")
                for kk_ in range(4):
                    nc.tensor.matmul(ps, ogT[kk_], wo_sb[kk_][:, nt * 512:(nt + 1) * 512],
                                     start=(kk_ == 0), stop=(kk_ == 3))
                nc.scalar.copy(outp[:, nt * 512:(nt + 1) * 512], ps)
            nc.sync.dma_start(op_in[c * 128:(c + 1) * 128, :], outp)
            # staged ReduceScatter: chunks 0-5 overlap the remaining compute,
            # chunks 6-7 form a short tail
            if c == 5:
                nc.gpsimd.collective_compute(
                    "ReduceScatter", OP.add,
                    replica_groups=[list(range(NCORES))],
                    ins=[op_in[0:768, :].opt()],
                    outs=[rs_out[0:96, :].opt()],
                )
            elif c == 7:
                nc.gpsimd.collective_compute(
                    "ReduceScatter", OP.add,
                    replica_groups=[list(range(NCORES))],
                    ins=[op_in[768:1024, :].opt()],
                    outs=[rs_out[96:128, :].opt()],
                )
        nc.sync.dma_start(d_out, rs_out[:])

    nc.compile()
    return nc


_BUILT = None


def _get_built():
    global _BUILT
    if _BUILT is None:
        _BUILT = build()
    return _BUILT


def _prep_in_maps(hidden_states, Wq, Wk, Wv, Wb, Wa, Wg, Wo,
                  conv_wq, conv_wk, conv_wv, A_log, dt_bias, norm_w):
    hsT = np.ascontiguousarray(hidden_states[0].astype(np.float32).T).astype(BF)
    Wq_b, Wk_b = Wq.astype(BF), Wk.astype(BF)
    Wv_b, Wg_b = Wv.astype(BF), Wg.astype(BF)
    Wb_b, Wa_b = Wb.astype(BF), Wa.astype(BF)
    Wo_eff = (np.tile(norm_w.astype(np.float32), H)[:, None] * Wo).astype(BF)
    cq = conv_wq.astype(np.float32)
    ck = conv_wk.astype(np.float32)
    cv = conv_wv.astype(np.float32)

    ii, jj = np.indices((128, 128))
    mask_a = np.where(jj < ii, 0.0, NEG).astype(np.float32)   # strict lower keep
    mask_m = np.where(jj >= ii, 0.0, NEG).astype(np.float32)  # upper incl diag keep
    masks = np.ascontiguousarray(np.concatenate([mask_a, mask_m], axis=1))
    identb = np.eye(128, dtype=np.float32).astype(BF)
    identf = np.eye(128, dtype=np.float32)
    onesf = np.ones((1, 128), np.float32)

    in_maps = []
    for core in range(NCORES):
        if core < 4:
            heads = [2 * core, 2 * core + 1]
            real = [True, True]
        else:
            heads = [4 + core, 4 + core]      # second slot is a dummy copy
            real = [True, False]
        qk_idx = np.concatenate([np.arange(h * DK, (h + 1) * DK) for h in heads])
        v_idx = np.concatenate([np.arange(h * DV, (h + 1) * DV) for h in heads])
        w = np.concatenate([Wq_b[:, qk_idx], Wk_b[:, qk_idx],
                            Wv_b[:, v_idx], Wg_b[:, v_idx],
                            Wb_b[:, heads], Wa_b[:, heads]], axis=1)
        convw = np.ascontiguousarray(
            np.concatenate([cq[qk_idx], ck[qk_idx], cv[v_idx]], axis=0))
        scal = np.stack([-np.exp(A_log[heads].astype(np.float32)),
                         dt_bias[heads].astype(np.float32)], axis=1)
        wo = Wo_eff[v_idx].copy()
        for j in range(NH):
            if not real[j]:
                wo[j * DV:(j + 1) * DV] = 0
        in_maps.append({
            "hsT": hsT, "w": np.ascontiguousarray(w),
            "wo": np.ascontiguousarray(wo),
            "convw": convw, "masks": masks, "identb": identb,
            "identf": identf, "onesf": onesf,
            "scal": np.ascontiguousarray(scal),
        })
    return in_maps


def _run(in_maps, trace=False):
    nc = _get_built()
    return run_bass_kernel_spmd(nc, in_maps, list(range(NCORES)), trace=trace)


def _assemble(res):
    # two ReduceScatter regions: rows 0:768 (96 rows/core) and 768:1024
    # (32 rows/core)
    out = np.zeros((T, HID), np.float32)
    for r in range(NCORES):
        o = np.asarray(res.results[r]["out"]).astype(np.float32)
        out[r * 96:(r + 1) * 96] = o[0:96]
        out[768 + r * 32:768 + (r + 1) * 32] = o[96:128]
    return out.reshape(1, T, HID)


def kernel(**inputs):
    in_maps = _prep_in_maps(**inputs)
    res = _run(in_maps, trace=False)
    return _assemble(res)


def _ensure_ntff_hook():
    import sys as _sys
    import types as _types
    try:
        from antenv.axon_hooks import get_axon_ntff_profile_hook  # noqa: F401
        return
    except ImportError:
        pass
    from trn_agent_boot.trn_boot import _ntff_profile_via_ctypes
    hook = _ntff_profile_via_ctypes('/opt/axon/libaxon_pjrt.so')
    mod = _types.ModuleType("antenv.axon_hooks")
    mod.get_axon_ntff_profile_hook = lambda: hook
    mod.set_axon_ntff_profile_hook = lambda h: None
    _sys.modules["antenv.axon_hooks"] = mod
    import antenv
    antenv.axon_hooks = mod
    import concourse.bass_utils as _bu
    _bu.upload_artifacts = lambda d: ""


def kernel_traced(**inputs):
    _ensure_ntff_hook()
    in_maps = _prep_in_maps(**inputs)
    res = _run(in_maps, trace=True)
    return _assemble(res), res
